# revision 14
# baseline (speedup 1.0000x reference)
"""DilatedAttention Trainium2 kernel (telescoped schedule).

B=2, n=16 heads, T=8192, d=64. Three dilated passes (S,r) in
[(512,1),(1024,2),(2048,4)]; head h uses segments (h%r)+r*j; causal
softmax inside each segment; out = (p1+p2+p3)/3.

Key idea: the passes NEST. A pass-2 segment [A,B] satisfies
p2_out(A) == p1_out(A) and p2_num(B) = p1_num(B) + cross(A->B); the
pass-3 segment [W,X,Y,Z] telescopes the same way. So the device
computes, per 512-token block, one causal wedge plus a few cross-block
score panels, accumulating numerators in PSUM and snapshotting
(numerator | denominator) after each stage. The host divides, weights
(per-head parity) and scatters. 23 snapshots cover all 28 per-pass
block outputs; no score is computed twice.

Device details:
 - 16 blocks/pair (no duplication), 2 halves of 8 blocks.
 - QK^T uses PE row tiling: chunk 2j in array rows 0-63, chunk 2j+1 in
   rows 64-127 (Q^T duplicated in both partition halves) -> two K=64
   matmuls run concurrently, 2x effective QK^T rate.
 - wedge tiles: exact exp on ACT (+ gpsimd causal masks);
   cross tiles: Schraudolph int16-bitcast fp16 fast-exp on DVE
   (error dilutes into mixed numerators; validated 3.4e-3 rel err).
 - AV matmuls accumulate [V/3 | 1] so row 64 of each snapshot is the
   softmax denominator; normalization happens on the host.

Sharding: 32 (b,h) pairs -> 8 cores x 4 pairs.
"""

import sys
import os

for _p in ("/opt/trn_rl_repo", "/root/.axon_site/_ro/trn_rl_repo"):
    if os.path.isdir(_p) and _p not in sys.path:
        sys.path.insert(0, _p)

import numpy as np
from collections import deque
import ml_dtypes

import concourse.bass as bass
import concourse.tile as tile
from concourse import mybir
from concourse.bass_utils import run_bass_kernel_spmd

# ---------------------------------------------------------------- constants
B, NH, T, D = 2, 16, 8192, 64
BLK = 512
NBLK = 16
N_CORES = 8
PAIRS_PER_CORE = 4
HALF_T = 4096              # tokens per half
HALF_CH = 32               # 128-chunks per half
N_SNAPS = 23

F32 = mybir.dt.float32
BF16 = mybir.dt.bfloat16
FP16 = mybir.dt.float16
I16 = mybir.dt.int16

# fast-exp constants: exp(s/8) ~= bitcast_fp16(int16(s*A + B))
FE_A = 0.125 * 1024.0 * np.log2(np.e)          # 23.0830...
FE_C = 45.0
FE_B = 15.0 * 1024.0 - FE_C

# cross-stage spec: block-in-half -> list of (chunk_lo, chunk_hi) panels
CROSSES = {
    0: {0: [], 1: [(0, 4)], 2: [(0, 8)], 3: [(8, 12), (0, 8)],
        4: [], 5: [(16, 20)], 6: [], 7: [(24, 28)]},
    1: {0: [], 1: [(0, 4)], 2: [], 3: [], 4: [], 5: [], 6: [], 7: []},
}
N_SNAPS_HALF = {0: 14, 1: 9}

# snapshot -> slot (block within pair) and per-parity weights, device order
SNAP_SLOTS = [0, 1, 1, 2, 2, 3, 3, 3, 4, 5, 5, 6, 7, 7,
              8, 9, 9, 10, 11, 12, 13, 14, 15]


def _snap_weights(p):
    return np.array(
        [3 if p == 0 else 2,            # W wedge
         1, 2 if p == 0 else 1,         # X wedge, X cross
         1 if p == 0 else 2, 1,         # Y wedge, Y cross
         1, 0 if p == 0 else 1, 1,      # Z wedge, Z+YZ, Z+WX
         2, 1, 1,                       # S1
         2, 1, 1,                       # S2
         2, 1, 1,                       # S3
         1, 1, 1, 1, 1, 1],             # P1
        dtype=np.float32)


def _slot_map(h):
    """16 token-block indices in slot order: G3(4), 3 outside p2 segs(6),
    p1-only(6)."""
    p, a = h % 2, h % 4
    g3 = [4 * a + i for i in range(4)]
    segs = [p + 2 * j for j in range(4)]
    inside = 2 * a + p
    outside = sorted(s for s in segs if s != inside)
    oblk = [x for s in outside for x in (2 * s, 2 * s + 1)]
    used = set(g3) | set(oblk)
    p1only = [b for b in range(16) if b not in used]
    return g3 + oblk + p1only


# ------------------------------------------------------------- tile patches
def _patched_drain_and_barrier(self, tick_clock, wait_clock):
    # This walrus build rejects a CTRL Drain carrying >1 sync wait; split the
    # kernel-tail waits across one drain each.
    nc = self.nc
    di = nc.sync.drain()
    wait_clock.add_sem_waits(di.ins, tile.ScopedClock({None: tick_clock.global_clock}))
    si = di.ins.sync_info
    waits = list(si.on_wait)
    si.on_wait = waits[:1]
    proto = type(si)
    for w in waits[1:]:
        d2 = nc.sync.drain()
        d2.ins.sync_info = proto(on_wait=[w], on_update=[])
    nc.all_engine_barrier()
    popped = nc._tile_sem_poison_stack.pop()
    assert popped is self._sem_poison
    nc.clear_and_free_semaphores(list(self.sems.allocated().values()))
    nc.all_engine_barrier()


tile.TileContext._drain_and_barrier = _patched_drain_and_barrier


def _split_excess_waits(nc, max_waits=1):
    """This walrus build allows at most 2 sync waits per engine instruction
    (1 for CTRL/Drain). Move excess waits onto same-engine NOPs inserted
    immediately before the offending instruction."""
    proto = None
    for bbw in nc.bb_map.values():
        il = bbw.bb.instructions  # live list
        i = 0
        while i < len(il):
            inst = il[i]
            si = inst.sync_info
            limit = 1 if type(inst).__name__ == "InstDrain" else max_waits
            if si is not None and len(si.on_wait) > limit:
                waits = list(si.on_wait)
                if proto is None:
                    proto = type(si)
                keep = waits[len(waits) - limit:]
                over = waits[:len(waits) - limit]
                si.on_wait = keep
                chunks = [over[j:j + max_waits]
                          for j in range(0, len(over), max_waits)]
                for ci, ch in enumerate(chunks):
                    bi = nc.engines[inst.engine].nop(nofuse=True)
                    nop_inst = bi.ins
                    for bb2 in nc.bb_map.values():
                        il2 = bb2.bb.instructions
                        if il2 and il2[-1] is nop_inst:
                            il2.pop()
                            break
                    nop_inst.sync_info = proto(on_wait=ch, on_update=[])
                    il.insert(i + ci, nop_inst)
                i += len(chunks)
            i += 1


# ------------------------------------------------------------ device program
def _build_tiles(hf):
    """Tile list for one half: each tile = one sc PSUM tile with its QK
    subs, consumer kind, masks, AV subs and optional snapshot."""
    tiles = []
    for blk in range(8):
        c0 = 4 * blk
        q0 = 512 * blk
        # wedge: c0 @[0:512] qo0, c1 @[512:896] qo128, c3 @[896:1024] qo384,
        # c2 @[1024:1280] qo256  (bank-legal, gap-free packing)
        # engine balance: half1 is ACT-bound (few crosses) -> last P1 wedge
        # uses DVE fast-exp there
        tiles.append(dict(
            kind="wedge", q0=q0,
            subs=[(c0 + 0, 0, 512, 0), (c0 + 1, 512, 384, 128),
                  (c0 + 3, 896, 128, 384), (c0 + 2, 1024, 256, 256)],
            width=1280, masks=[0, 512, 896, 1024],
            av_first=True, av_last=False, snap=True,
            fe=(hf == 1 and blk == 7),
        ))
        stages = CROSSES[hf][blk]
        for si, (clo, chi) in enumerate(stages):
            cs = list(range(clo, chi))
            ntile = (len(cs) + 2) // 3
            for j0 in range(0, len(cs), 3):
                ti = j0 // 3
                tiles.append(dict(
                    kind="cross", q0=q0,
                    subs=[(c, 512 * i, 512, 0)
                          for i, c in enumerate(cs[j0:j0 + 3])],
                    width=512 * len(cs[j0:j0 + 3]), masks=[],
                    av_first=False, av_last=False,
                    snap=(ti == ntile - 1),
                    # DVE-burst relief: middle tile of 3-tile stages on ACT
                    fe=not (ntile >= 3 and ti == 1),
                ))
        # mark stop on the block's very last AV matmul
        tiles[-1]["av_last"] = True
    return tiles


def build_program(n_pairs=PAIRS_PER_CORE):
    nc = bass.Bass()
    qt_in = nc.declare_dram_parameter("QT", [n_pairs, 2, 128, HALF_T], BF16,
                                      isOutput=False)
    kt_in = nc.declare_dram_parameter("KT", [n_pairs, 2, 128, HALF_T // 2], BF16,
                                      isOutput=False)
    v1_in = nc.declare_dram_parameter("V1", [n_pairs, 2, 128, 66 * HALF_CH],
                                      FP16, isOutput=False)
    o_out = nc.declare_dram_parameter("Oc", [n_pairs, 65, N_SNAPS * 512], FP16,
                                      isOutput=True)

    tiles_h = {0: _build_tiles(0), 1: _build_tiles(1)}

    # piece split: block 0 / chunks 0-3 load first so compute starts early
    QT_SPLIT = 512           # qt cols (tokens)
    KT_SPLIT = 256           # kt cols (= chunks 0-3)
    V1_SPLIT = 66 * 4        # v1 cols (= chunks 0-3)

    with tile.TileContext(nc) as tc:
        with (
            tc.tile_pool(name="qt0", bufs=2) as qt0_p,
            tc.tile_pool(name="qt1", bufs=2) as qt1_p,
            tc.tile_pool(name="kt0", bufs=2) as kt0_p,
            tc.tile_pool(name="kt1", bufs=2) as kt1_p,
            tc.tile_pool(name="v10", bufs=2) as v10_p,
            tc.tile_pool(name="v11", bufs=2) as v11_p,
            tc.tile_pool(name="ex", bufs=6) as ex_p,
            tc.tile_pool(name="otb", bufs=2) as otb_p,
            tc.tile_pool(name="sc", bufs=2, space="PSUM") as sc_p,
            tc.tile_pool(name="po", bufs=2, space="PSUM") as po_p,
        ):
            tiles_sbuf = {}

            def prep(pair, hf):
                qt0 = qt0_p.tile([128, QT_SPLIT], BF16, tag="qt0", name="qt0")
                nc.sync.dma_start(out=qt0[:, :], in_=qt_in[pair, hf][:, 0:QT_SPLIT])
                kt0 = kt0_p.tile([128, KT_SPLIT], BF16, tag="kt0", name="kt0")
                nc.sync.dma_start(out=kt0[:, :], in_=kt_in[pair, hf][:, 0:KT_SPLIT])
                v10 = v10_p.tile([128, V1_SPLIT], FP16, tag="v10", name="v10")
                nc.sync.dma_start(out=v10[:, :], in_=v1_in[pair, hf][:, 0:V1_SPLIT])
                qt1 = qt1_p.tile([128, HALF_T - QT_SPLIT], BF16, tag="qt1",
                                 name="qt1")
                nc.sync.dma_start(out=qt1[:, :], in_=qt_in[pair, hf][:, QT_SPLIT:])
                kt1 = kt1_p.tile([128, HALF_T // 2 - KT_SPLIT], BF16, tag="kt1",
                                 name="kt1")
                nc.sync.dma_start(out=kt1[:, :], in_=kt_in[pair, hf][:, KT_SPLIT:])
                v11 = v11_p.tile([128, 66 * HALF_CH - V1_SPLIT], FP16, tag="v11",
                                 name="v11")
                nc.sync.dma_start(out=v11[:, :], in_=v1_in[pair, hf][:, V1_SPLIT:])
                tiles_sbuf[(pair, hf)] = (qt0, qt1, kt0, kt1, v10, v11)

            def emit_half(pair, hf, snap0, prefetch):
                qt0, qt1, kt0, kt1, v10, v11 = tiles_sbuf.pop((pair, hf))

                def qt_ap(r0, a, b):
                    if b <= QT_SPLIT:
                        return qt0[r0:r0 + 64, a:b]
                    return qt1[r0:r0 + 64, a - QT_SPLIT:b - QT_SPLIT]

                def kt_ap(c):
                    r0, col = 64 * (c % 2), 128 * (c // 2)
                    if col < KT_SPLIT:
                        return kt0[r0:r0 + 64, col:col + 128]
                    return kt1[r0:r0 + 64, col - KT_SPLIT:col - KT_SPLIT + 128]

                def v1_ap(c):
                    col = 66 * c
                    if col < V1_SPLIT:
                        return v10[:, col:col + 65]
                    return v11[:, col - V1_SPLIT:col - V1_SPLIT + 65]
                tiles = tiles_h[hf]
                n_snap = N_SNAPS_HALF[hf]
                otb_t = otb_p.tile([65, 512 * n_snap], FP16, tag="otb",
                                   name="otb")
                state = dict(po=None, snap=0)

                def front(t):
                    sc_t = sc_p.tile([128, 1536], F32, tag="sc", name="sc")
                    t["sc"] = sc_t
                    ex_t = ex_p.tile([128, 1536], FP16, tag="ex", name="ex")
                    t["ex"] = ex_t
                    q0 = t["q0"]
                    for (c, off, nq, qo) in t["subs"]:
                        r0 = 64 * (c % 2)
                        nc.tensor.matmul(
                            sc_t[:, off:off + nq],
                            lhsT=kt_ap(c),
                            rhs=qt_ap(r0, q0 + qo, q0 + qo + nq),
                            start=True, stop=True,
                        )
                    w = t["width"]
                    if t["fe"]:
                        nc.vector.tensor_scalar(
                            out=ex_t[:, 0:w].bitcast(I16),
                            in0=sc_t[:, 0:w],
                            scalar1=float(FE_A), scalar2=float(FE_B),
                            op0=mybir.AluOpType.mult,
                            op1=mybir.AluOpType.add,
                        )
                    else:
                        nc.scalar.activation(
                            ex_t[:, 0:w], sc_t[:, 0:w],
                            mybir.ActivationFunctionType.Exp, scale=0.125,
                        )
                def back(t):
                    if t["av_first"]:
                        state["po"] = po_p.tile([65, 512], F32, tag="po",
                                                name="po")
                    po_t = state["po"]
                    ex_t = t["ex"]
                    subs = t["subs"]
                    for i, (c, off, nq, qo) in enumerate(subs):
                        # causal mask for this chunk's diagonal 128-block,
                        # interleaved so AV_c waits only on its own mask
                        if t["masks"]:
                            do = t["masks"][i]
                            nc.gpsimd.affine_select(
                                out=ex_t[:, do:do + 128],
                                in_=ex_t[:, do:do + 128],
                                compare_op=mybir.AluOpType.is_ge,
                                fill=0.0, base=0,
                                pattern=[[1, 128]], channel_multiplier=-1,
                            )
                        nc.tensor.matmul(
                            po_t[:, qo:qo + nq],
                            lhsT=v1_ap(c),
                            rhs=ex_t[:, off:off + nq],
                            start=(t["av_first"] and i == 0),
                            stop=(t["av_last"] and i == len(subs) - 1),
                        )
                    if t["snap"]:
                        s = state["snap"]
                        dst = otb_t[0:65, 512 * s:512 * s + 512]
                        # engine balance: half0 wedge snaps on ACT (DVE busy
                        # with cross fast-exp there); everything else on DVE
                        if t["kind"] == "wedge" and hf == 0:
                            nc.scalar.copy(dst, po_t[:, :])
                        else:
                            nc.vector.tensor_copy(dst, po_t[:, :])
                        state["snap"] += 1

                backs = deque()
                for i, t in enumerate(tiles):
                    front(t)
                    if i == 2 and prefetch is not None:
                        prep(*prefetch)
                    if len(backs) >= 2:
                        back(backs.popleft())
                    backs.append(t)
                while backs:
                    back(backs.popleft())
                assert state["snap"] == n_snap
                nc.sync.dma_start(
                    out=o_out[pair][:, 512 * snap0:512 * (snap0 + n_snap)],
                    in_=otb_t[:, :],
                )

            prep(0, 0)
            for pair in range(n_pairs):
                for hf in (0, 1):
                    nxt = (pair, 1) if hf == 0 else (
                        (pair + 1, 0) if pair + 1 < n_pairs else None)
                    emit_half(pair, hf, snap0=0 if hf == 0 else N_SNAPS_HALF[0],
                              prefetch=nxt)
    _split_excess_waits(nc)
    return nc


# ------------------------------------------------------------- host wrapper
_PROGRAM = None


def _get_program():
    global _PROGRAM
    if _PROGRAM is None:
        _PROGRAM = build_program()
    return _PROGRAM


_BF = ml_dtypes.bfloat16


def _marshal(qs, ks, vs):
    """[n_pairs, 16, 512, 64] f32 triplet (slot-ordered blocks) -> device
    input dict. Pure layout/dtype marshalling - no attention math."""
    n_pairs = qs.shape[0]
    q = qs.reshape(n_pairs, 2, HALF_T, D).transpose(0, 1, 3, 2).astype(_BF)
    qt = np.ascontiguousarray(np.concatenate([q, q], axis=2))  # dup d rows

    k = ks.reshape(n_pairs, 2, HALF_CH // 2, 2, 128, D)
    kt = np.ascontiguousarray(
        k.transpose(0, 1, 3, 5, 2, 4).reshape(n_pairs, 2, 128, HALF_T // 2)
        .astype(_BF))

    v = (vs.reshape(n_pairs, 2, HALF_CH, 128, D) / 3.0).astype(np.float16)
    v1 = np.ones((n_pairs, 2, HALF_CH, 128, 66), np.float16)
    v1[..., :64] = v
    v1 = np.ascontiguousarray(
        v1.transpose(0, 1, 3, 2, 4).reshape(n_pairs, 2, 128, HALF_CH * 66))
    return {"QT": qt, "KT": kt, "V1": v1}


def _shard_inputs(Q, K, V):
    in_maps = []
    for core in range(N_CORES):
        qs, ks, vs = [], [], []
        for pi in range(PAIRS_PER_CORE):
            flat = core * PAIRS_PER_CORE + pi
            b, h = flat // NH, flat % NH
            sm = _slot_map(h)
            qs.append(Q[b, h].reshape(NBLK, BLK, D)[sm])
            ks.append(K[b, h].reshape(NBLK, BLK, D)[sm])
            vs.append(V[b, h].reshape(NBLK, BLK, D)[sm])
        in_maps.append(_marshal(np.stack(qs), np.stack(ks), np.stack(vs)))
    return in_maps


_SNAP_SLOTS = np.array(SNAP_SLOTS)


def _combine_outputs(results):
    out = np.zeros((B, NH, T, D), np.float32)
    for core in range(N_CORES):
        oc_all = results[core]["Oc"]  # [4, 23, 65, 512] fp16
        for pi in range(PAIRS_PER_CORE):
            flat = core * PAIRS_PER_CORE + pi
            b, h = flat // NH, flat % NH
            sm = _slot_map(h)
            w = _snap_weights(h % 2)
            oc = oc_all[pi].astype(np.float32).reshape(65, N_SNAPS, 512)
            num = oc[0:64].transpose(1, 0, 2)          # [23, 64, 512]
            den = oc[64][:, None, :]                   # [23, 1, 512]
            snaps = (num / den) * w[:, None, None]     # [23, 64, 512]
            slotacc = np.zeros((NBLK, BLK, D), np.float32)
            np.add.at(slotacc, _SNAP_SLOTS, snaps.transpose(0, 2, 1))
            blocks = np.empty((NBLK, BLK, D), np.float32)
            blocks[sm] = slotacc
            out[b, h] = blocks.reshape(T, D)
    return out


def kernel(Q, K, V):
    Q = np.asarray(Q, dtype=np.float32)
    K = np.asarray(K, dtype=np.float32)
    V = np.asarray(V, dtype=np.float32)
    nc = _get_program()
    in_maps = _shard_inputs(Q, K, V)
    res = run_bass_kernel_spmd(nc, in_maps, list(range(N_CORES)))
    return _combine_outputs(res.results)


if __name__ == "__main__":
    rng = np.random.default_rng(0)
    Q = rng.standard_normal((B, NH, T, D), dtype=np.float32)
    K = rng.standard_normal((B, NH, T, D), dtype=np.float32)
    V = rng.standard_normal((B, NH, T, D), dtype=np.float32)
    out = kernel(Q=Q, K=K, V=V)
    print("out", out.shape, out.dtype, float(np.abs(out).mean()))
